# revision 21
# baseline (speedup 1.0000x reference)
"""DiffAttention (differential attention) TRN2 Bass kernel.

Problem: nn_DiffAttention_15977278341927
  B=2, N=2048, DIM=1024, 16 heads of dim 64 -> 8 effective head-pairs.
  out = ((softmax(Q1K1^T) - lam*softmax(Q2K2^T)) @ V) -> headwise RMSNorm
        -> *(1-lam_init) -> concat heads -> @ Wo + bo

Sharding (8 cores): data-parallel over batch (2) x head-parallel over
effective-head pairs (4 groups of 2 pairs).  Core c handles batch c//4 and
pairs {2*(c%4), 2*(c%4)+1}.  QKV weights column-sharded, Wo row-sharded,
partial outputs summed on host (the unshard reduction).

Device dataflow is entirely in "feature-on-partition / token-on-free" space:
  xT = x^T (host pre-transpose) -> QT,KT (d-pair on partitions) -> S^T tiles
  (keys on partitions) -> exp on ScalarE (PSUM->SBUF, the only O(N^2)
  non-PE op) -> PV and ones-row matmuls on PE -> per-token scalars
  (softmax denominators, rms) as [1,n] rows, replicated across partitions
  via a DRAM-bounce broadcast DMA -> output projection consumes outn^T
  directly -> y^T partial written to DRAM; host transposes + sums.

Softmax normalization is folded past the PV matmul (scale-invariance of
RMSNorm): out ~ normalize(U1 - lam*(s1/s2)*U2), avoiding any O(N^2)
elementwise work on the vector engine.
"""

import os
import sys

sys.path.insert(0, "/opt/trn_rl_repo")

import ml_dtypes
import numpy as np

import concourse.bacc as bacc
import concourse.bass as bass
import concourse.mybir as mybir
import concourse.tile as tile
from concourse.masks import make_identity

B, N, DIM = 2, 2048, 1024
NUM_HEADS = 16
EFF = 8
HEAD_DIM = 64
D2 = 2 * HEAD_DIM  # 128, one head-pair's q/k cols and one v head's dims
LAMBDA_INIT = 0.8
EPS = 1e-5

P = 128
CH = 512  # n-chunk (queries per chunk)
NCH = N // CH  # 4
MT = N // P  # 16 key tiles
KT_D = DIM // P  # 8 contraction tiles over DIM
PAIRS = 2  # pairs per core
CORES = 8

F32 = mybir.dt.float32
F32R = mybir.dt.float32r
BF16 = mybir.dt.bfloat16
DEBUG = bool(int(os.environ.get("KERNEL_DEBUG", "0")))
HOT = BF16


def _r(ap):
    return ap


def _h(ap):
    return ap


def _emit(tc, t, iters=1):
    """Emit the per-core program.  t: dict of dram tensor APs."""
    from contextlib import ExitStack

    nc = tc.nc
    Act = mybir.ActivationFunctionType
    Alu = mybir.AluOpType

    ctx = ExitStack()
    with ctx:
        singles = ctx.enter_context(tc.tile_pool(name="singles", bufs=1))
        big = ctx.enter_context(tc.tile_pool(name="big", bufs=2))
        work = ctx.enter_context(tc.tile_pool(name="work", bufs=2))
        ypool = ctx.enter_context(tc.tile_pool(name="ypool", bufs=2))
        expp = ctx.enter_context(tc.tile_pool(name="expp", bufs=3))
        reps = ctx.enter_context(tc.tile_pool(name="reps", bufs=3))
        rows = ctx.enter_context(tc.tile_pool(name="rows", bufs=4))
        pa = ctx.enter_context(tc.tile_pool(name="pa", bufs=2, space="PSUM"))
        pb = ctx.enter_context(tc.tile_pool(name="pb", bufs=4, space="PSUM"))
        dscr = ctx.enter_context(tc.tile_pool(name="dscr", bufs=2, space="DRAM"))

        # ---- loads / constants ----
        xt_sb = singles.tile([P, KT_D, N], BF16)
        nc.sync.dma_start(xt_sb, t["xt"].rearrange("(k p) n -> p k n", p=P))
        w_sb = {}
        for w in ("wq", "wk", "wv"):
            w_sb[w] = singles.tile([P, KT_D, PAIRS * D2], BF16, name=f"w_{w}")
            nc.sync.dma_start(w_sb[w], t[w].rearrange("(k p) c -> p k c", p=P))
        wo_sb = singles.tile([P, PAIRS, DIM], BF16)
        nc.sync.dma_start(wo_sb, t["wo"].rearrange("(u p) c -> p u c", p=P))
        params_sb = singles.tile([P, 8], F32)
        nc.gpsimd.dma_start(params_sb, t["params"])
        ones_hot = singles.tile([P, 1], HOT)
        nc.vector.memset(ones_hot, 1.0)
        ident = singles.tile([P, P], HOT)
        make_identity(nc, ident)

        outnT = [singles.tile([P, N], BF16, name=f"outnT{u}") for u in range(PAIRS)]

        import contextlib

        loop_cm = (
            tc.For_i(0, iters, 1, hint_engines=(mybir.EngineType.PE,))
            if iters > 1
            else contextlib.nullcontext()
        )
        with loop_cm:
            _emit_body(tc, t, locals())


def _emit_body(tc, t, env):
    nc = tc.nc
    Act = mybir.ActivationFunctionType
    Alu = mybir.AluOpType
    singles = env["singles"]; big = env["big"]; work = env["work"]
    ypool = env["ypool"]; expp = env["expp"]; reps = env["reps"]; rows = env["rows"]
    pa = env["pa"]; pb = env["pb"]; dscr = env["dscr"]
    xt_sb = env["xt_sb"]; w_sb = env["w_sb"]; wo_sb = env["wo_sb"]
    params_sb = env["params_sb"]; ones_hot = env["ones_hot"]; ident = env["ident"]
    outnT = env["outnT"]

    if True:
        for u in range(PAIRS):
            usl = slice(u * D2, (u + 1) * D2)

            # ---- phase 1: projections (QT, KT [d-pair, n]; V [m, d]) ----
            QT = big.tile([P, N], HOT, tag="QT")
            KTt = big.tile([P, N], HOT, tag="KTt")
            V_sb = big.tile([P, MT, P], HOT, tag="V")
            for nch in range(NCH):
                sl = slice(nch * CH, (nch + 1) * CH)
                ps = pa.tile([P, 2, CH], F32, tag="pa")
                for kt in range(KT_D):
                    nc.tensor.matmul(
                        ps[:, 0, :],
                        lhsT=_r(w_sb["wq"][:, kt, usl]),
                        rhs=_r(xt_sb[:, kt, sl]),
                        start=(kt == 0),
                        stop=(kt == KT_D - 1),
                    )
                for kt in range(KT_D):
                    nc.tensor.matmul(
                        ps[:, 1, :],
                        lhsT=_r(w_sb["wk"][:, kt, usl]),
                        rhs=_r(xt_sb[:, kt, sl]),
                        start=(kt == 0),
                        stop=(kt == KT_D - 1),
                    )
                nc.scalar.activation(
                    QT[:, sl], ps[:, 0, :], Act.Identity,
                    bias=params_sb[:, u : u + 1],
                )
                nc.scalar.activation(
                    KTt[:, sl], ps[:, 1, :], Act.Identity,
                    bias=params_sb[:, 2 + u : 3 + u],
                )

                psv = pb.tile([P, CH], F32, tag="pb")
                for kt in range(KT_D):
                    nc.tensor.matmul(
                        psv,
                        lhsT=_r(w_sb["wv"][:, kt, usl]),
                        rhs=_r(xt_sb[:, kt, sl]),
                        start=(kt == 0),
                        stop=(kt == KT_D - 1),
                    )
                vtc = work.tile([P, CH], HOT, tag="vtc")
                nc.scalar.activation(
                    vtc, psv, Act.Identity, bias=params_sb[:, 4 + u : 5 + u]
                )
                for i in range(CH // P):
                    mt = nch * (CH // P) + i
                    pst = pb.tile([P, CH], HOT, tag="pb")
                    nc.tensor.transpose(pst[:, :P], vtc[:, i * P : (i + 1) * P], ident)
                    nc.vector.tensor_copy(V_sb[:, mt, :], pst[:, :P])

            if DEBUG and u == 0:
                nc.sync.dma_start(t["dbg_qt"], QT)
                nc.sync.dma_start(t["dbg_kt"], KTt)
                nc.sync.dma_start(t["dbg_v"], V_sb.rearrange("p a b -> p (a b)"))

            # ---- phase 2: S^T -> exp -> PV + denominators, per n-chunk ----
            for nch in range(NCH):
                sl = slice(nch * CH, (nch + 1) * CH)
                U1 = pb.tile([P, CH], F32, tag="pb", name=f"U1_{u}_{nch}")
                U2 = pb.tile([P, CH], F32, tag="pb", name=f"U2_{u}_{nch}")
                S1s = pb.tile([P, CH], F32, tag="pb", name=f"s1_{u}_{nch}")
                S2s = pb.tile([P, CH], F32, tag="pb", name=f"s2_{u}_{nch}")
                for mt in range(MT):
                    msl = slice(mt * P, (mt + 1) * P)
                    sp = pa.tile([P, 2, CH], F32, tag="pa")
                    nc.tensor.matmul(sp[:, 0, :], lhsT=_h(KTt[0:64, msl]), rhs=_h(QT[0:64, sl]))
                    nc.tensor.matmul(
                        sp[:, 1, :], lhsT=_h(KTt[64:128, msl]), rhs=_h(QT[64:128, sl])
                    )
                    ex = expp.tile([P, 2, CH], HOT, tag="ex")
                    nc.scalar.activation(ex, sp, Act.Exp)
                    nc.tensor.matmul(
                        U1,
                        lhsT=_h(V_sb[:, mt, :]),
                        rhs=_h(ex[:, 0, :]),
                        start=(mt == 0),
                        stop=(mt == MT - 1),
                    )
                    nc.tensor.matmul(
                        U2,
                        lhsT=_h(V_sb[:, mt, :]),
                        rhs=_h(ex[:, 1, :]),
                        start=(mt == 0),
                        stop=(mt == MT - 1),
                    )
                    nc.tensor.matmul(
                        S1s[0:1, :],
                        lhsT=_h(ones_hot),
                        rhs=_h(ex[:, 0, :]),
                        start=(mt == 0),
                        stop=(mt == MT - 1),
                    )
                    nc.tensor.matmul(
                        S2s[0:1, :],
                        lhsT=_h(ones_hot),
                        rhs=_h(ex[:, 1, :]),
                        start=(mt == 0),
                        stop=(mt == MT - 1),
                    )

                # f[n] = lam * s1[n] / s2[n]
                s1r = rows.tile([1, CH], F32, tag="row", name="s1r")
                nc.vector.tensor_copy(s1r, S1s[0:1, :])
                t_row = rows.tile([1, CH], F32, tag="row", name="t_row")
                nc.vector.reciprocal(t_row, S2s[0:1, :])
                f_row = rows.tile([1, CH], F32, tag="row", name="f_row")
                nc.vector.tensor_tensor(f_row, s1r, t_row, Alu.mult)
                nc.vector.tensor_scalar_mul(f_row, f_row, params_sb[0:1, 7:8])
                dr = dscr.tile([1, CH], F32, tag="d1")
                nc.sync.dma_start(dr, f_row)
                Frep = reps.tile([P, 1, CH], F32, tag="rep", name="Frep")
                nc.gpsimd.dma_start(Frep, dr[:].partition_broadcast(P))

                if DEBUG and u == 0:
                    nc.sync.dma_start(t["dbg_f"][:, sl], f_row)

                # Udiff = U1 - f*U2   (all [128, CH])
                u2f = work.tile([P, CH], F32, tag="u2f")
                nc.vector.tensor_tensor(u2f, U2, Frep[:, 0, :], Alu.mult)
                ud = work.tile([P, CH], F32, tag="ud")
                nc.vector.tensor_tensor(ud, U1, u2f, Alu.subtract)

                # rmsinv[n] = exp(-0.5 * ln(mean_d(ud^2) + eps))
                sq = work.tile([P, CH], BF16, tag="sq")
                nc.vector.tensor_mul(sq, ud, ud)
                ssq = pb.tile([P, CH], F32, tag="pb", name=f"ssq_{u}_{nch}")
                nc.tensor.matmul(ssq[0:1, :], lhsT=ones_hot, rhs=sq)
                # ud is s1-scaled; fold eps exactly: mean(o^2)+eps =
                # (ssq + 128*eps*s1^2) / 128 / s1^2, and the s1^2 cancels
                # against ud's scale inside the normalize.
                sq1 = rows.tile([1, CH], F32, tag="row", name="sq1")
                nc.vector.tensor_tensor(sq1, s1r, s1r, Alu.mult)
                adj = rows.tile([1, CH], F32, tag="row", name="adj")
                nc.vector.scalar_tensor_tensor(
                    out=adj,
                    in0=sq1,
                    scalar=float(P) * EPS,
                    in1=ssq[0:1, :],
                    op0=Alu.mult,
                    op1=Alu.add,
                )
                lnr = rows.tile([1, CH], F32, tag="row", name="lnr")
                nc.scalar.activation(lnr, adj, Act.Ln, scale=1.0 / P)
                rir = rows.tile([1, CH], F32, tag="row", name="rir")
                nc.scalar.activation(rir, lnr, Act.Exp, scale=-0.5)
                if DEBUG and u == 0:
                    nc.sync.dma_start(t["dbg_ud"][:, sl], ud)
                dr2 = dscr.tile([1, CH], F32, tag="d2")
                nc.sync.dma_start(dr2, rir)
                Rrep = reps.tile([P, 1, CH], F32, tag="rep", name="Rrep")
                nc.gpsimd.dma_start(Rrep, dr2[:].partition_broadcast(P))

                if DEBUG and u == 0:
                    nc.sync.dma_start(t["dbg_rinv"][:, sl], rir)
                # outn^T = (ud * geff) * rmsinv
                nc.vector.scalar_tensor_tensor(
                    out=outnT[u][:, sl],
                    in0=ud,
                    scalar=params_sb[:, 6:7],
                    in1=Rrep[:, 0, :],
                    op0=Alu.mult,
                    op1=Alu.mult,
                )

        if DEBUG:
            nc.sync.dma_start(t["dbg_onT"], outnT[0])
            nc.sync.dma_start(t["dbg_onT1"], outnT[1])

        # ---- phase 3: output projection, y^T partial [DIM, N] ----
        for dt in range(DIM // P):
            dsl = slice(dt * P, (dt + 1) * P)
            for nch in range(NCH):
                sl = slice(nch * CH, (nch + 1) * CH)
                yp = pb.tile([P, CH], F32, tag="pb", name=f"y_{dt}_{nch}")
                for u in range(PAIRS):
                    nc.tensor.matmul(
                        yp,
                        lhsT=_r(wo_sb[:, u, dsl]),
                        rhs=_r(outnT[u][:, sl]),
                        start=(u == 0),
                        stop=(u == PAIRS - 1),
                    )
                ysb = ypool.tile([P, CH], F32, tag="ysb")
                if (dt + nch) % 2 == 0:
                    nc.vector.tensor_copy(ysb, yp)
                else:
                    nc.scalar.copy(ysb, yp)
                nc.sync.dma_start(t["ypart"][dsl, sl], ysb)


def build_program(iters=1):
    nc = bacc.Bacc("TRN2", target_bir_lowering=False, debug=False)
    t = {
        "xt": nc.dram_tensor("xt", [DIM, N], BF16, kind="ExternalInput")[:],
        "wq": nc.dram_tensor("wq", [DIM, PAIRS * D2], BF16, kind="ExternalInput")[:],
        "wk": nc.dram_tensor("wk", [DIM, PAIRS * D2], BF16, kind="ExternalInput")[:],
        "wv": nc.dram_tensor("wv", [DIM, PAIRS * D2], BF16, kind="ExternalInput")[:],
        "wo": nc.dram_tensor("wo", [PAIRS * D2, DIM], BF16, kind="ExternalInput")[:],
        "params": nc.dram_tensor("params", [P, 8], F32, kind="ExternalInput")[:],
        "ypart": nc.dram_tensor("ypart", [DIM, N], F32, kind="ExternalOutput")[:],
    }
    if DEBUG:
        for nm, shp in [
            ("dbg_qt", [P, N]), ("dbg_kt", [P, N]), ("dbg_v", [P, MT * P]),
            ("dbg_f", [1, N]), ("dbg_rinv", [1, N]), ("dbg_ud", [P, N]),
            ("dbg_onT", [P, N]), ("dbg_onT1", [P, N]),
        ]:
            t[nm] = nc.dram_tensor(nm, shp, F32, kind="ExternalOutput")[:]
    with tile.TileContext(nc) as tc:
        _emit(tc, t, iters=iters)
    nc.compile()
    return nc


_NC_CACHE = {}


def _get_nc(iters=1):
    if iters not in _NC_CACHE:
        _NC_CACHE[iters] = build_program(iters)
    return _NC_CACHE[iters]


def make_core_inputs(x, Wq, bq, Wk, bk, Wv, bv, Wo, bo, g, lq1, lk1, lq2, lk2):
    """Host-side shard prep.  Returns (in_maps, lam) for the 8 cores."""
    x = np.asarray(x, np.float32)
    scaling = np.float32(HEAD_DIM**-0.5)
    lam1 = np.exp(np.sum(np.asarray(lq1, np.float32) * np.asarray(lk1, np.float32)))
    lam2 = np.exp(np.sum(np.asarray(lq2, np.float32) * np.asarray(lk2, np.float32)))
    lam = np.float32(lam1 - lam2 + LAMBDA_INIT)

    xt = np.ascontiguousarray(np.transpose(x, (0, 2, 1)))  # (B, DIM, N)
    Wq_s = np.asarray(Wq, np.float32) * scaling
    bq_s = np.asarray(bq, np.float32) * scaling
    geff = np.ascontiguousarray(
        (np.asarray(g, np.float32) * np.float32(1.0 - LAMBDA_INIT)).reshape(P, 1)
    )

    in_maps = []
    for c in range(CORES):
        b = c // 4
        grp = c % 4
        cols = slice(grp * PAIRS * D2, (grp + 1) * PAIRS * D2)
        params = np.zeros((P, 8), np.float32)
        params[:, 0:2] = bq_s[cols].reshape(PAIRS, P).T
        params[:, 2:4] = np.asarray(bk, np.float32)[cols].reshape(PAIRS, P).T
        params[:, 4:6] = np.asarray(bv, np.float32)[cols].reshape(PAIRS, P).T
        params[:, 6] = geff[:, 0]
        params[:, 7] = lam
        in_maps.append(
            {
                "xt": np.ascontiguousarray(xt[b]).astype(ml_dtypes.bfloat16),
                "wq": np.ascontiguousarray(Wq_s[:, cols]).astype(ml_dtypes.bfloat16),
                "wk": np.ascontiguousarray(np.asarray(Wk, np.float32)[:, cols]).astype(
                    ml_dtypes.bfloat16
                ),
                "wv": np.ascontiguousarray(np.asarray(Wv, np.float32)[:, cols]).astype(
                    ml_dtypes.bfloat16
                ),
                "wo": np.ascontiguousarray(np.asarray(Wo, np.float32)[cols, :]).astype(
                    ml_dtypes.bfloat16
                ),
                "params": params,
            }
        )
    return in_maps, lam


def gather_output(results, bo):
    """Sum per-core y^T partials per batch, transpose, add bo."""
    bo = np.asarray(bo, np.float32)
    out = np.empty((B, N, DIM), np.float32)
    for b in range(B):
        acc = np.zeros((DIM, N), np.float32)
        for c in range(b * 4, b * 4 + 4):
            acc += results[c]["ypart"]
        out[b] = acc.T + bo
    return out


_IN_CACHE = {}


def kernel(**inputs):
    from concourse.bass_utils import run_bass_kernel_spmd

    key = id(inputs.get("x"))
    if key in _IN_CACHE:
        in_maps = _IN_CACHE[key]
    else:
        in_maps, _ = make_core_inputs(**inputs)
        _IN_CACHE.clear()
        _IN_CACHE[key] = in_maps
    iters = int(os.environ.get("KERNEL_ITERS", "1"))
    nc = _get_nc(iters)
    trace = bool(int(os.environ.get("KERNEL_TRACE", "0")))
    res = run_bass_kernel_spmd(
        nc, in_maps, core_ids=list(range(CORES)), trace=trace
    )
    if trace and res.exec_time_ns is not None:
        print(f"HW exec time: {res.exec_time_ns} ns")
        kernel.last_exec_time_ns = res.exec_time_ns
        kernel.last_trace = res.instructions_and_trace
    return gather_output(res.results, inputs["bo"])


# ---------------- dev helpers (not used by the grading harness) ----------------


def _numpy_core_partial(im):
    """Reference computation of one core's ypart from its sharded inputs."""
    xt = im["xt"].astype(np.float64)  # [DIM, N]
    x = xt.T
    pr = im["params"]
    lam = float(pr[0, 7])
    ypart = np.zeros((DIM, N))
    for u in range(PAIRS):
        usl = slice(u * D2, (u + 1) * D2)
        q = x @ im["wq"][:, usl].astype(np.float64) + pr[:, u]  # [N, 128]
        k = x @ im["wk"][:, usl].astype(np.float64) + pr[:, 2 + u]
        v = x @ im["wv"][:, usl].astype(np.float64) + pr[:, 4 + u]
        s1 = q[:, :64] @ k[:, :64].T
        s2 = q[:, 64:] @ k[:, 64:].T
        p1 = np.exp(s1)
        p1 /= p1.sum(-1, keepdims=True)
        p2 = np.exp(s2)
        p2 /= p2.sum(-1, keepdims=True)
        diff = p1 - lam * p2
        o = diff @ v  # [N, 128]
        rms = 1.0 / np.sqrt((o * o).mean(-1, keepdims=True) + EPS)
        o = o * rms * pr[:, 6]
        ypart += im["wo"][usl, :].astype(np.float64).T @ o.T
    return ypart


if __name__ == "__main__":
    # CoreSim single-core numerical check:  python kernel.py sim
    mode = sys.argv[1] if len(sys.argv) > 1 else "sim"
    sys.path.insert(0, "/root/problem")
    import reference

    inputs = {k: np.asarray(v) for k, v in reference.setup_inputs().items()}
    in_maps, lam = make_core_inputs(**inputs)
    print("lam =", lam)
    nc = _get_nc()
    print("program built:", len(nc.inst_map) if hasattr(nc, "inst_map") else "?")
    if mode == "sim":
        from concourse.bass_interp import CoreSim

        sim = CoreSim(nc)
        for k, v in in_maps[0].items():
            sim.tensor(k)[:] = v
        sim.simulate()
        got = np.array(sim.tensor("ypart"))
        want = _numpy_core_partial(in_maps[0])
        err = np.abs(got - want)
        scale = np.abs(want).max()
        print("absmax err:", err.max(), "rel:", err.max() / scale, "scale:", scale)
        try:
            print("sim predicted time:", sim.time, "ns")
        except Exception as e:
            print("no sim time:", e)


# revision 30
# speedup vs baseline: 1.9143x; 1.9143x over previous
"""DiffAttention (differential attention) TRN2 Bass kernel.

Problem: nn_DiffAttention_15977278341927
  B=2, N=2048, DIM=1024, 16 heads of dim 64 -> 8 effective head-pairs.
  out = ((softmax(Q1K1^T) - lam*softmax(Q2K2^T)) @ V) -> headwise RMSNorm
        -> *(1-lam_init) -> concat heads -> @ Wo + bo

Sharding (8 cores): data-parallel over batch (2) x head-parallel over
effective-head pairs (4 groups of 2 pairs).  Core c handles batch c//4 and
pairs {2*(c%4), 2*(c%4)+1}.  QKV weights column-sharded, Wo row-sharded,
partial outputs summed on host (the unshard reduction).

Device dataflow is entirely in "feature-on-partition / token-on-free" space:
  xT = x^T (host pre-transpose) -> QT,KT (d-pair on partitions) -> S^T tiles
  (keys on partitions) -> exp on ScalarE (PSUM->SBUF, the only O(N^2)
  non-PE op) -> PV and ones-row matmuls on PE -> per-token scalars
  (softmax denominators, rms) as [1,n] rows, replicated across partitions
  via gpsimd partition_broadcast -> output projection consumes outn^T
  directly -> y^T partial written to DRAM; host transposes + sums.

Softmax normalization is folded past the PV matmul (scale-invariance of
RMSNorm): out ~ normalize(U1 - lam*(s1/s2)*U2), avoiding any O(N^2)
elementwise work on the vector engine.
"""

import os
import sys

sys.path.insert(0, "/opt/trn_rl_repo")

import ml_dtypes
import numpy as np

import concourse.bacc as bacc
import concourse.bass as bass
import concourse.mybir as mybir
import concourse.tile as tile
from concourse.masks import make_identity

B, N, DIM = 2, 2048, 1024
NUM_HEADS = 16
EFF = 8
HEAD_DIM = 64
D2 = 2 * HEAD_DIM  # 128, one head-pair's q/k cols and one v head's dims
LAMBDA_INIT = 0.8
EPS = 1e-5

P = 128
CH = 512  # n-chunk (queries per chunk)
NCH = N // CH  # 4
MT = N // P  # 16 key tiles
KT_D = DIM // P  # 8 contraction tiles over DIM
PAIRS = 2  # pairs per core
CORES = 8

F32 = mybir.dt.float32
F32R = mybir.dt.float32r
BF16 = mybir.dt.bfloat16
DEBUG = bool(int(os.environ.get("KERNEL_DEBUG", "0")))
ABL = os.environ.get("KERNEL_ABL", "")  # ablations for timing experiments
HOT = BF16


def _r(ap):
    return ap


def _h(ap):
    return ap


def _emit(tc, t, iters=1):
    """Emit the per-core program.  t: dict of dram tensor APs."""
    from contextlib import ExitStack

    nc = tc.nc
    Act = mybir.ActivationFunctionType
    Alu = mybir.AluOpType

    ctx = ExitStack()
    with ctx:
        singles = ctx.enter_context(tc.tile_pool(name="singles", bufs=1))
        big = ctx.enter_context(tc.tile_pool(name="big", bufs=2))
        work = ctx.enter_context(tc.tile_pool(name="work", bufs=2))
        ypool = ctx.enter_context(tc.tile_pool(name="ypool", bufs=2))
        expp = ctx.enter_context(tc.tile_pool(name="expp", bufs=6))
        reps = ctx.enter_context(tc.tile_pool(name="reps", bufs=3))
        rows = ctx.enter_context(tc.tile_pool(name="rows", bufs=4))
        rowsN = ctx.enter_context(tc.tile_pool(name="rowsN", bufs=3))
        repsN = ctx.enter_context(tc.tile_pool(name="repsN", bufs=2))
        work2 = ctx.enter_context(tc.tile_pool(name="work2", bufs=2))
        pa = ctx.enter_context(tc.tile_pool(name="pa", bufs=2, space="PSUM"))
        pb = ctx.enter_context(tc.tile_pool(name="pb", bufs=4, space="PSUM"))

        # ---- loads / constants ----
        xt_sb = singles.tile([P, KT_D, N], BF16)
        nc.sync.dma_start(xt_sb, t["xt"].rearrange("(k p) n -> p k n", p=P))
        w_sb = {}
        for w in ("wq", "wk", "wv"):
            w_sb[w] = singles.tile([P, KT_D, PAIRS * D2], BF16, name=f"w_{w}")
            nc.sync.dma_start(w_sb[w], t[w].rearrange("(k p) c -> p k c", p=P))
        wo_sb = singles.tile([P, PAIRS, DIM], BF16)
        nc.sync.dma_start(wo_sb, t["wo"].rearrange("(u p) c -> p u c", p=P))
        params_sb = singles.tile([P, 8], F32)
        nc.gpsimd.dma_start(params_sb, t["params"])
        ones_hot = singles.tile([P, 1], HOT)
        nc.vector.memset(ones_hot, 1.0)
        ones_r = singles.tile([P, 1], F32R)
        nc.vector.tensor_copy(ones_r, ones_hot)
        ident_b = singles.tile([P, P], HOT)
        make_identity(nc, ident_b)
        ident = singles.tile([P, P], F32R)
        nc.vector.tensor_copy(ident, ident_b)

        outnT = [singles.tile([P, N], BF16, name=f"outnT{u}") for u in range(PAIRS)]

        import contextlib

        loop_cm = (
            tc.For_i(0, iters, 1, hint_engines=(mybir.EngineType.PE,))
            if iters > 1
            else contextlib.nullcontext()
        )
        with loop_cm:
            _emit_body(tc, t, locals())


def _emit_body(tc, t, env):
    nc = tc.nc
    Act = mybir.ActivationFunctionType
    Alu = mybir.AluOpType
    singles = env["singles"]; big = env["big"]; work = env["work"]
    ypool = env["ypool"]; expp = env["expp"]; reps = env["reps"]; rows = env["rows"]
    rowsN = env["rowsN"]; repsN = env["repsN"]; work2 = env["work2"]
    pa = env["pa"]; pb = env["pb"]
    xt_sb = env["xt_sb"]; w_sb = env["w_sb"]; wo_sb = env["wo_sb"]
    params_sb = env["params_sb"]; ones_hot = env["ones_hot"]; ident = env["ident"]
    ones_r = env["ones_r"]
    outnT = env["outnT"]

    if True:
        for u in range(PAIRS):
            usl = slice(u * D2, (u + 1) * D2)

            # ---- phase 1: projections (QT, KT [d-pair, n]; V [m, d]) ----
            QT = big.tile([P, N], HOT, tag="QT")
            KTt = big.tile([P, N], HOT, tag="KTt")
            V_sb = big.tile([P, MT, P], F32R, tag="V")
            for nch in range(NCH):
                sl = slice(nch * CH, (nch + 1) * CH)
                ps = pa.tile([P, 2, CH], F32, tag="pa")
                for kt in range(KT_D):
                    nc.tensor.matmul(
                        ps[:, 0, :],
                        lhsT=_r(w_sb["wq"][:, kt, usl]),
                        rhs=_r(xt_sb[:, kt, sl]),
                        start=(kt == 0),
                        stop=(kt == KT_D - 1),
                    )
                for kt in range(KT_D):
                    nc.tensor.matmul(
                        ps[:, 1, :],
                        lhsT=_r(w_sb["wk"][:, kt, usl]),
                        rhs=_r(xt_sb[:, kt, sl]),
                        start=(kt == 0),
                        stop=(kt == KT_D - 1),
                    )
                nc.scalar.activation(
                    QT[:, sl], ps[:, 0, :], Act.Identity,
                    bias=params_sb[:, u : u + 1],
                )
                nc.scalar.activation(
                    KTt[:, sl], ps[:, 1, :], Act.Identity,
                    bias=params_sb[:, 2 + u : 3 + u],
                )

                psv = pb.tile([P, CH], F32, tag="pb")
                for kt in range(KT_D):
                    nc.tensor.matmul(
                        psv,
                        lhsT=_r(w_sb["wv"][:, kt, usl]),
                        rhs=_r(xt_sb[:, kt, sl]),
                        start=(kt == 0),
                        stop=(kt == KT_D - 1),
                    )
                vtc = work.tile([P, CH], F32R, tag="vtc")
                nc.scalar.activation(
                    vtc, psv, Act.Identity, bias=params_sb[:, 4 + u : 5 + u]
                )
                for i in range(CH // P):
                    mt = nch * (CH // P) + i
                    pst = pb.tile([P, CH], F32R, tag="pb")
                    nc.tensor.transpose(pst[:, :P], vtc[:, i * P : (i + 1) * P], ident)
                    nc.vector.tensor_copy(V_sb[:, mt, :], pst[:, :P])

            if DEBUG and u == 0:
                nc.sync.dma_start(t["dbg_qt"], QT)
                nc.sync.dma_start(t["dbg_kt"], KTt)
                nc.sync.dma_start(t["dbg_v"], V_sb.rearrange("p a b -> p (a b)"))

            # ---- phase 2: S^T -> exp -> PV + denominators, per n-chunk ----
            udbuf = work2.tile([P, N], F32, tag="udb", name=f"udb{u}")
            adjb = rowsN.tile([1, N], F32, tag="rowN", name=f"adjb{u}")
            for nch in range(NCH):
                sl = slice(nch * CH, (nch + 1) * CH)
                U1 = pb.tile([P, CH], F32, tag="pb", name=f"U1_{u}_{nch}")
                U2 = pb.tile([P, CH], F32, tag="pb", name=f"U2_{u}_{nch}")
                S1s = pb.tile([P, CH], F32, tag="pb", name=f"s1_{u}_{nch}")
                S2s = pb.tile([P, CH], F32, tag="pb", name=f"s2_{u}_{nch}")
                for mt in range(MT):
                    msl = slice(mt * P, (mt + 1) * P)
                    sp = pa.tile([P, 2, CH], F32, tag="pa")
                    nc.tensor.matmul(sp[:, 0, :], lhsT=_h(KTt[0:64, msl]), rhs=_h(QT[0:64, sl]))
                    nc.tensor.matmul(
                        sp[:, 1, :], lhsT=_h(KTt[64:128, msl]), rhs=_h(QT[64:128, sl])
                    )
                    ex = expp.tile([P, 2, CH], F32R, tag="ex")
                    if ABL == "noexp":
                        nc.vector.tensor_copy(ex, sp)
                    else:
                        nc.scalar.activation(ex, sp, Act.Exp)
                    nc.tensor.matmul(
                        U1,
                        lhsT=_h(V_sb[:, mt, :]),
                        rhs=_h(ex[:, 0, :]),
                        start=(mt == 0),
                        stop=(mt == MT - 1),
                    )
                    nc.tensor.matmul(
                        U2,
                        lhsT=_h(V_sb[:, mt, :]),
                        rhs=_h(ex[:, 1, :]),
                        start=(mt == 0),
                        stop=(mt == MT - 1),
                    )
                    if ABL != "nodenom":
                        nc.tensor.matmul(
                            S1s[0:1, :],
                            lhsT=_h(ones_r),
                            rhs=_h(ex[:, 0, :]),
                            start=(mt == 0),
                            stop=(mt == MT - 1),
                        )
                        nc.tensor.matmul(
                            S2s[0:1, :],
                            lhsT=_h(ones_r),
                            rhs=_h(ex[:, 1, :]),
                            start=(mt == 0),
                            stop=(mt == MT - 1),
                        )

                if ABL == "nodenom":
                    nc.vector.memset(S1s[0:1, :], 2000.0)
                    nc.vector.memset(S2s[0:1, :], 2000.0)
                SKIP_ROWS = ABL == "norows"
                # f[n] = lam * s1[n] / s2[n]
                Frep = reps.tile([P, 1, CH], F32, tag="rep", name="Frep")
                if SKIP_ROWS:
                    nc.vector.memset(Frep, 0.95)
                    s1r = None
                else:
                    s1r = rows.tile([1, CH], F32, tag="row", name="s1r")
                    nc.vector.tensor_copy(s1r, S1s[0:1, :])
                    t_row = rows.tile([1, CH], F32, tag="row", name="t_row")
                    nc.vector.reciprocal(t_row, S2s[0:1, :])
                    f_row = rows.tile([1, CH], F32, tag="row", name="f_row")
                    nc.vector.tensor_tensor(f_row, s1r, t_row, Alu.mult)
                    nc.vector.tensor_scalar_mul(f_row, f_row, params_sb[0:1, 7:8])
                    nc.gpsimd.partition_broadcast(Frep[:, 0, :], f_row, channels=P)

                if DEBUG and u == 0:
                    nc.sync.dma_start(t["dbg_f"][:, sl], f_row)

                # Udiff = U1 - f*U2   (all [128, CH])
                u2f = work.tile([P, CH], F32, tag="u2f")
                nc.vector.tensor_tensor(u2f, U2, Frep[:, 0, :], Alu.mult)
                ud = udbuf[:, sl]
                nc.vector.tensor_tensor(ud, U1, u2f, Alu.subtract)

                # mean-square row for rmsinv, with the exact-eps fold:
                # ud is s1-scaled; mean(o^2)+eps = (ssq + 128*eps*s1^2)/128/s1^2
                # and the s1^2 cancels inside the normalize.
                sq = work.tile([P, CH], BF16, tag="sq")
                nc.vector.tensor_mul(sq, ud, ud)
                ssq = pb.tile([P, CH], F32, tag="pb", name=f"ssq_{u}_{nch}")
                nc.tensor.matmul(ssq[0:1, :], lhsT=ones_hot, rhs=sq)
                if not SKIP_ROWS:
                    sq1 = rows.tile([1, CH], F32, tag="row", name="sq1")
                    nc.vector.tensor_tensor(sq1, s1r, s1r, Alu.mult)
                    nc.vector.scalar_tensor_tensor(
                        out=adjb[0:1, sl],
                        in0=sq1,
                        scalar=float(P) * EPS,
                        in1=ssq[0:1, :],
                        op0=Alu.mult,
                        op1=Alu.add,
                    )
                else:
                    nc.vector.tensor_copy(adjb[0:1, sl], ssq[0:1, :])

            # per-unit batched rmsinv: one Ln + one Exp (same activation
            # table set stays loaded across the unit's 128 exp calls).
            lnr = rowsN.tile([1, N], F32, tag="rowN", name="lnr")
            nc.scalar.activation(lnr, adjb, Act.Ln, scale=1.0 / P)
            rir = rowsN.tile([1, N], F32, tag="rowN", name="rir")
            nc.scalar.activation(rir, lnr, Act.Exp, scale=-0.5)
            RrepN = repsN.tile([P, 1, N], F32, tag="repN", name="RrepN")
            nc.gpsimd.partition_broadcast(RrepN[:, 0, :], rir, channels=P)
            if DEBUG and u == 0:
                nc.sync.dma_start(t["dbg_ud"], udbuf)
                nc.sync.dma_start(t["dbg_rinv"], rir)
            # outn^T = (ud * geff) * rmsinv
            nc.vector.scalar_tensor_tensor(
                out=outnT[u],
                in0=udbuf,
                scalar=params_sb[:, 6:7],
                in1=RrepN[:, 0, :],
                op0=Alu.mult,
                op1=Alu.mult,
            )

        if DEBUG:
            nc.sync.dma_start(t["dbg_onT"], outnT[0])
            nc.sync.dma_start(t["dbg_onT1"], outnT[1])

        # ---- phase 3: output projection, y^T partial [DIM, N] ----
        for dt in ([] if ABL == "nop3" else range(DIM // P)):
            dsl = slice(dt * P, (dt + 1) * P)
            for nch in range(NCH):
                sl = slice(nch * CH, (nch + 1) * CH)
                yp = pb.tile([P, CH], F32, tag="pb", name=f"y_{dt}_{nch}")
                for u in range(PAIRS):
                    nc.tensor.matmul(
                        yp,
                        lhsT=_r(wo_sb[:, u, dsl]),
                        rhs=_r(outnT[u][:, sl]),
                        start=(u == 0),
                        stop=(u == PAIRS - 1),
                    )
                ysb = ypool.tile([P, CH], F32, tag="ysb")
                if (dt + nch) % 2 == 0:
                    nc.vector.tensor_copy(ysb, yp)
                else:
                    nc.scalar.copy(ysb, yp)
                nc.sync.dma_start(t["ypart"][dsl, sl], ysb)


def build_program(iters=1):
    nc = bacc.Bacc("TRN2", target_bir_lowering=False, debug=False)
    t = {
        "xt": nc.dram_tensor("xt", [DIM, N], BF16, kind="ExternalInput")[:],
        "wq": nc.dram_tensor("wq", [DIM, PAIRS * D2], BF16, kind="ExternalInput")[:],
        "wk": nc.dram_tensor("wk", [DIM, PAIRS * D2], BF16, kind="ExternalInput")[:],
        "wv": nc.dram_tensor("wv", [DIM, PAIRS * D2], BF16, kind="ExternalInput")[:],
        "wo": nc.dram_tensor("wo", [PAIRS * D2, DIM], BF16, kind="ExternalInput")[:],
        "params": nc.dram_tensor("params", [P, 8], F32, kind="ExternalInput")[:],
        "ypart": nc.dram_tensor("ypart", [DIM, N], F32, kind="ExternalOutput")[:],
    }
    if DEBUG:
        for nm, shp in [
            ("dbg_qt", [P, N]), ("dbg_kt", [P, N]), ("dbg_v", [P, MT * P]),
            ("dbg_f", [1, N]), ("dbg_rinv", [1, N]), ("dbg_ud", [P, N]),
            ("dbg_onT", [P, N]), ("dbg_onT1", [P, N]),
        ]:
            t[nm] = nc.dram_tensor(nm, shp, F32, kind="ExternalOutput")[:]
    with tile.TileContext(nc) as tc:
        _emit(tc, t, iters=iters)
    nc.compile()
    return nc


_NC_CACHE = {}


def _get_nc(iters=1):
    if iters not in _NC_CACHE:
        _NC_CACHE[iters] = build_program(iters)
    return _NC_CACHE[iters]


def make_core_inputs(x, Wq, bq, Wk, bk, Wv, bv, Wo, bo, g, lq1, lk1, lq2, lk2):
    """Host-side shard prep.  Returns (in_maps, lam) for the 8 cores."""
    x = np.asarray(x, np.float32)
    scaling = np.float32(HEAD_DIM**-0.5)
    lam1 = np.exp(np.sum(np.asarray(lq1, np.float32) * np.asarray(lk1, np.float32)))
    lam2 = np.exp(np.sum(np.asarray(lq2, np.float32) * np.asarray(lk2, np.float32)))
    lam = np.float32(lam1 - lam2 + LAMBDA_INIT)

    xt = np.ascontiguousarray(np.transpose(x, (0, 2, 1)))  # (B, DIM, N)
    Wq_s = np.asarray(Wq, np.float32) * scaling
    bq_s = np.asarray(bq, np.float32) * scaling
    geff = np.ascontiguousarray(
        (np.asarray(g, np.float32) * np.float32(1.0 - LAMBDA_INIT)).reshape(P, 1)
    )

    in_maps = []
    for c in range(CORES):
        b = c // 4
        grp = c % 4
        cols = slice(grp * PAIRS * D2, (grp + 1) * PAIRS * D2)
        params = np.zeros((P, 8), np.float32)
        params[:, 0:2] = bq_s[cols].reshape(PAIRS, P).T
        params[:, 2:4] = np.asarray(bk, np.float32)[cols].reshape(PAIRS, P).T
        params[:, 4:6] = np.asarray(bv, np.float32)[cols].reshape(PAIRS, P).T
        params[:, 6] = geff[:, 0]
        params[:, 7] = lam
        in_maps.append(
            {
                "xt": np.ascontiguousarray(xt[b]).astype(ml_dtypes.bfloat16),
                "wq": np.ascontiguousarray(Wq_s[:, cols]).astype(ml_dtypes.bfloat16),
                "wk": np.ascontiguousarray(np.asarray(Wk, np.float32)[:, cols]).astype(
                    ml_dtypes.bfloat16
                ),
                "wv": np.ascontiguousarray(np.asarray(Wv, np.float32)[:, cols]).astype(
                    ml_dtypes.bfloat16
                ),
                "wo": np.ascontiguousarray(np.asarray(Wo, np.float32)[cols, :]).astype(
                    ml_dtypes.bfloat16
                ),
                "params": params,
            }
        )
    return in_maps, lam


def gather_output(results, bo):
    """Sum per-core y^T partials per batch, transpose, add bo."""
    bo = np.asarray(bo, np.float32)
    out = np.empty((B, N, DIM), np.float32)
    for b in range(B):
        acc = np.zeros((DIM, N), np.float32)
        for c in range(b * 4, b * 4 + 4):
            acc += results[c]["ypart"]
        out[b] = acc.T + bo
    return out


_IN_CACHE = {}


def kernel(**inputs):
    from concourse.bass_utils import run_bass_kernel_spmd

    key = id(inputs.get("x"))
    if key in _IN_CACHE:
        in_maps = _IN_CACHE[key]
    else:
        in_maps, _ = make_core_inputs(**inputs)
        _IN_CACHE.clear()
        _IN_CACHE[key] = in_maps
    iters = int(os.environ.get("KERNEL_ITERS", "1"))
    nc = _get_nc(iters)
    trace = bool(int(os.environ.get("KERNEL_TRACE", "0")))
    res = run_bass_kernel_spmd(
        nc, in_maps, core_ids=list(range(CORES)), trace=trace
    )
    if trace and res.exec_time_ns is not None:
        print(f"HW exec time: {res.exec_time_ns} ns")
        kernel.last_exec_time_ns = res.exec_time_ns
        kernel.last_trace = res.instructions_and_trace
    return gather_output(res.results, inputs["bo"])


# ---------------- dev helpers (not used by the grading harness) ----------------


def _numpy_core_partial(im):
    """Reference computation of one core's ypart from its sharded inputs."""
    xt = im["xt"].astype(np.float64)  # [DIM, N]
    x = xt.T
    pr = im["params"]
    lam = float(pr[0, 7])
    ypart = np.zeros((DIM, N))
    for u in range(PAIRS):
        usl = slice(u * D2, (u + 1) * D2)
        q = x @ im["wq"][:, usl].astype(np.float64) + pr[:, u]  # [N, 128]
        k = x @ im["wk"][:, usl].astype(np.float64) + pr[:, 2 + u]
        v = x @ im["wv"][:, usl].astype(np.float64) + pr[:, 4 + u]
        s1 = q[:, :64] @ k[:, :64].T
        s2 = q[:, 64:] @ k[:, 64:].T
        p1 = np.exp(s1)
        p1 /= p1.sum(-1, keepdims=True)
        p2 = np.exp(s2)
        p2 /= p2.sum(-1, keepdims=True)
        diff = p1 - lam * p2
        o = diff @ v  # [N, 128]
        rms = 1.0 / np.sqrt((o * o).mean(-1, keepdims=True) + EPS)
        o = o * rms * pr[:, 6]
        ypart += im["wo"][usl, :].astype(np.float64).T @ o.T
    return ypart


if __name__ == "__main__":
    # CoreSim single-core numerical check:  python kernel.py sim
    mode = sys.argv[1] if len(sys.argv) > 1 else "sim"
    sys.path.insert(0, "/root/problem")
    import reference

    inputs = {k: np.asarray(v) for k, v in reference.setup_inputs().items()}
    in_maps, lam = make_core_inputs(**inputs)
    print("lam =", lam)
    nc = _get_nc()
    print("program built:", len(nc.inst_map) if hasattr(nc, "inst_map") else "?")
    if mode == "sim":
        from concourse.bass_interp import CoreSim

        sim = CoreSim(nc)
        for k, v in in_maps[0].items():
            sim.tensor(k)[:] = v
        sim.simulate()
        got = np.array(sim.tensor("ypart"))
        want = _numpy_core_partial(in_maps[0])
        err = np.abs(got - want)
        scale = np.abs(want).max()
        print("absmax err:", err.max(), "rel:", err.max() / scale, "scale:", scale)
        try:
            print("sim predicted time:", sim.time, "ns")
        except Exception as e:
            print("no sim time:", e)


# revision 31
# speedup vs baseline: 2.0530x; 1.0725x over previous
"""DiffAttention (differential attention) TRN2 Bass kernel.

Problem: nn_DiffAttention_15977278341927
  B=2, N=2048, DIM=1024, 16 heads of dim 64 -> 8 effective head-pairs.
  out = ((softmax(Q1K1^T) - lam*softmax(Q2K2^T)) @ V) -> headwise RMSNorm
        -> *(1-lam_init) -> concat heads -> @ Wo + bo

Sharding (8 cores): data-parallel over batch (2) x head-parallel over
effective-head pairs (4 groups of 2 pairs).  Core c handles batch c//4 and
pairs {2*(c%4), 2*(c%4)+1}.  QKV weights column-sharded, Wo row-sharded,
partial outputs summed on host (the unshard reduction).

Device dataflow is entirely in "feature-on-partition / token-on-free" space:
  xT = x^T (host pre-transpose) -> QT,KT (d-pair on partitions) -> S^T tiles
  (keys on partitions) -> exp on ScalarE (PSUM->SBUF, the only O(N^2)
  non-PE op) -> PV and ones-row matmuls on PE -> per-token scalars
  (softmax denominators, rms) as [1,n] rows, replicated across partitions
  via gpsimd partition_broadcast -> output projection consumes outn^T
  directly -> y^T partial written to DRAM; host transposes + sums.

Softmax normalization is folded past the PV matmul (scale-invariance of
RMSNorm): out ~ normalize(U1 - lam*(s1/s2)*U2), avoiding any O(N^2)
elementwise work on the vector engine.
"""

import os
import sys

sys.path.insert(0, "/opt/trn_rl_repo")

import ml_dtypes
import numpy as np

import concourse.bacc as bacc
import concourse.bass as bass
import concourse.mybir as mybir
import concourse.tile as tile
from concourse.masks import make_identity

B, N, DIM = 2, 2048, 1024
NUM_HEADS = 16
EFF = 8
HEAD_DIM = 64
D2 = 2 * HEAD_DIM  # 128, one head-pair's q/k cols and one v head's dims
LAMBDA_INIT = 0.8
EPS = 1e-5

P = 128
CH = 512  # n-chunk (queries per chunk)
NCH = N // CH  # 4
MT = N // P  # 16 key tiles
KT_D = DIM // P  # 8 contraction tiles over DIM
PAIRS = 2  # pairs per core
CORES = 8

F32 = mybir.dt.float32
F32R = mybir.dt.float32r
BF16 = mybir.dt.bfloat16
DEBUG = bool(int(os.environ.get("KERNEL_DEBUG", "0")))
ABL = os.environ.get("KERNEL_ABL", "")  # ablations for timing experiments
HOT = BF16


def _r(ap):
    return ap


def _h(ap):
    return ap


def _emit(tc, t, iters=1):
    """Emit the per-core program.  t: dict of dram tensor APs."""
    from contextlib import ExitStack

    nc = tc.nc
    Act = mybir.ActivationFunctionType
    Alu = mybir.AluOpType

    ctx = ExitStack()
    with ctx:
        singles = ctx.enter_context(tc.tile_pool(name="singles", bufs=1))
        big = ctx.enter_context(tc.tile_pool(name="big", bufs=2))
        work = ctx.enter_context(tc.tile_pool(name="work", bufs=2))
        ypool = ctx.enter_context(tc.tile_pool(name="ypool", bufs=2))
        expp = ctx.enter_context(tc.tile_pool(name="expp", bufs=8))
        reps = ctx.enter_context(tc.tile_pool(name="reps", bufs=3))
        rows = ctx.enter_context(tc.tile_pool(name="rows", bufs=4))
        rowsN = ctx.enter_context(tc.tile_pool(name="rowsN", bufs=3))
        repsN = ctx.enter_context(tc.tile_pool(name="repsN", bufs=2))
        work2 = ctx.enter_context(tc.tile_pool(name="work2", bufs=2))
        pa = ctx.enter_context(tc.tile_pool(name="pa", bufs=2, space="PSUM"))
        pb = ctx.enter_context(tc.tile_pool(name="pb", bufs=4, space="PSUM"))

        # ---- loads / constants ----
        xt_sb = singles.tile([P, KT_D, N], BF16)
        nc.sync.dma_start(xt_sb, t["xt"].rearrange("(k p) n -> p k n", p=P))
        w_sb = {}
        for w in ("wq", "wk", "wv"):
            w_sb[w] = singles.tile([P, KT_D, PAIRS * D2], BF16, name=f"w_{w}")
            nc.sync.dma_start(w_sb[w], t[w].rearrange("(k p) c -> p k c", p=P))
        wo_sb = singles.tile([P, PAIRS, DIM], BF16)
        nc.sync.dma_start(wo_sb, t["wo"].rearrange("(u p) c -> p u c", p=P))
        params_sb = singles.tile([P, 8], F32)
        nc.gpsimd.dma_start(params_sb, t["params"])
        ones_hot = singles.tile([P, 1], HOT)
        nc.vector.memset(ones_hot, 1.0)
        ones_r = singles.tile([P, 1], F32R)
        nc.vector.tensor_copy(ones_r, ones_hot)
        ident_b = singles.tile([P, P], HOT)
        make_identity(nc, ident_b)
        ident = singles.tile([P, P], F32R)
        nc.vector.tensor_copy(ident, ident_b)

        outnT = [singles.tile([P, N], BF16, name=f"outnT{u}") for u in range(PAIRS)]

        import contextlib

        loop_cm = (
            tc.For_i(0, iters, 1, hint_engines=(mybir.EngineType.PE,))
            if iters > 1
            else contextlib.nullcontext()
        )
        with loop_cm:
            _emit_body(tc, t, locals())


def _emit_body(tc, t, env):
    nc = tc.nc
    Act = mybir.ActivationFunctionType
    Alu = mybir.AluOpType
    singles = env["singles"]; big = env["big"]; work = env["work"]
    ypool = env["ypool"]; expp = env["expp"]; reps = env["reps"]; rows = env["rows"]
    rowsN = env["rowsN"]; repsN = env["repsN"]; work2 = env["work2"]
    pa = env["pa"]; pb = env["pb"]
    xt_sb = env["xt_sb"]; w_sb = env["w_sb"]; wo_sb = env["wo_sb"]
    params_sb = env["params_sb"]; ones_hot = env["ones_hot"]; ident = env["ident"]
    ones_r = env["ones_r"]
    outnT = env["outnT"]

    if True:
        for u in range(PAIRS):
            usl = slice(u * D2, (u + 1) * D2)

            # ---- phase 1: projections (QT, KT [d-pair, n]; V [m, d]) ----
            QT = big.tile([P, N], HOT, tag="QT")
            KTt = big.tile([P, N], HOT, tag="KTt")
            V_sb = big.tile([P, MT, P], F32R, tag="V")
            for nch in range(NCH):
                sl = slice(nch * CH, (nch + 1) * CH)
                ps = pa.tile([P, 2, CH], F32, tag="pa")
                for kt in range(KT_D):
                    nc.tensor.matmul(
                        ps[:, 0, :],
                        lhsT=_r(w_sb["wq"][:, kt, usl]),
                        rhs=_r(xt_sb[:, kt, sl]),
                        start=(kt == 0),
                        stop=(kt == KT_D - 1),
                    )
                for kt in range(KT_D):
                    nc.tensor.matmul(
                        ps[:, 1, :],
                        lhsT=_r(w_sb["wk"][:, kt, usl]),
                        rhs=_r(xt_sb[:, kt, sl]),
                        start=(kt == 0),
                        stop=(kt == KT_D - 1),
                    )
                nc.scalar.activation(
                    QT[:, sl], ps[:, 0, :], Act.Identity,
                    bias=params_sb[:, u : u + 1],
                )
                nc.scalar.activation(
                    KTt[:, sl], ps[:, 1, :], Act.Identity,
                    bias=params_sb[:, 2 + u : 3 + u],
                )

                psv = pb.tile([P, CH], F32, tag="pb")
                for kt in range(KT_D):
                    nc.tensor.matmul(
                        psv,
                        lhsT=_r(w_sb["wv"][:, kt, usl]),
                        rhs=_r(xt_sb[:, kt, sl]),
                        start=(kt == 0),
                        stop=(kt == KT_D - 1),
                    )
                vtc = work.tile([P, CH], F32R, tag="vtc")
                nc.scalar.activation(
                    vtc, psv, Act.Identity, bias=params_sb[:, 4 + u : 5 + u]
                )
                for i in range(CH // P):
                    mt = nch * (CH // P) + i
                    pst = pb.tile([P, CH], F32R, tag="pb")
                    nc.tensor.transpose(pst[:, :P], vtc[:, i * P : (i + 1) * P], ident)
                    nc.vector.tensor_copy(V_sb[:, mt, :], pst[:, :P])

            if DEBUG and u == 0:
                nc.sync.dma_start(t["dbg_qt"], QT)
                nc.sync.dma_start(t["dbg_kt"], KTt)
                nc.sync.dma_start(t["dbg_v"], V_sb.rearrange("p a b -> p (a b)"))

            # ---- phase 2: S^T -> exp -> PV + denominators, per n-chunk ----
            udbuf = work2.tile([P, N], F32, tag="udb", name=f"udb{u}")
            adjb = rowsN.tile([1, N], F32, tag="rowN", name=f"adjb{u}")
            for nch in range(NCH):
                sl = slice(nch * CH, (nch + 1) * CH)
                U1 = pb.tile([P, CH], F32, tag="pb", name=f"U1_{u}_{nch}")
                U2 = pb.tile([P, CH], F32, tag="pb", name=f"U2_{u}_{nch}")
                S1s = pb.tile([P, CH], F32, tag="pb", name=f"s1_{u}_{nch}")
                S2s = pb.tile([P, CH], F32, tag="pb", name=f"s2_{u}_{nch}")
                for mt in range(MT):
                    msl = slice(mt * P, (mt + 1) * P)
                    sp = pa.tile([P, 2, CH], F32, tag="pa")
                    nc.tensor.matmul(sp[:, 0, :], lhsT=_h(KTt[0:64, msl]), rhs=_h(QT[0:64, sl]))
                    nc.tensor.matmul(
                        sp[:, 1, :], lhsT=_h(KTt[64:128, msl]), rhs=_h(QT[64:128, sl])
                    )
                    ex = expp.tile([P, 2, CH], F32R, tag="ex")
                    if ABL == "noexp":
                        nc.vector.tensor_copy(ex, sp)
                    else:
                        nc.scalar.activation(ex, sp, Act.Exp)
                    nc.tensor.matmul(
                        U1,
                        lhsT=_h(V_sb[:, mt, :]),
                        rhs=_h(ex[:, 0, :]),
                        start=(mt == 0),
                        stop=(mt == MT - 1),
                    )
                    nc.tensor.matmul(
                        U2,
                        lhsT=_h(V_sb[:, mt, :]),
                        rhs=_h(ex[:, 1, :]),
                        start=(mt == 0),
                        stop=(mt == MT - 1),
                    )
                    if ABL != "nodenom":
                        nc.tensor.matmul(
                            S1s[0:1, :],
                            lhsT=_h(ones_r),
                            rhs=_h(ex[:, 0, :]),
                            start=(mt == 0),
                            stop=(mt == MT - 1),
                        )
                        nc.tensor.matmul(
                            S2s[0:1, :],
                            lhsT=_h(ones_r),
                            rhs=_h(ex[:, 1, :]),
                            start=(mt == 0),
                            stop=(mt == MT - 1),
                        )

                if ABL == "nodenom":
                    nc.vector.memset(S1s[0:1, :], 2000.0)
                    nc.vector.memset(S2s[0:1, :], 2000.0)
                SKIP_ROWS = ABL == "norows"
                # f[n] = lam * s1[n] / s2[n]
                Frep = reps.tile([P, 1, CH], F32, tag="rep", name="Frep")
                if SKIP_ROWS:
                    nc.vector.memset(Frep, 0.95)
                    s1r = None
                else:
                    s1r = rows.tile([1, CH], F32, tag="row", name="s1r")
                    nc.vector.tensor_copy(s1r, S1s[0:1, :])
                    t_row = rows.tile([1, CH], F32, tag="row", name="t_row")
                    nc.vector.reciprocal(t_row, S2s[0:1, :])
                    f_row = rows.tile([1, CH], F32, tag="row", name="f_row")
                    nc.vector.tensor_tensor(f_row, s1r, t_row, Alu.mult)
                    nc.vector.tensor_scalar_mul(f_row, f_row, params_sb[0:1, 7:8])
                    nc.gpsimd.partition_broadcast(Frep[:, 0, :], f_row, channels=P)

                if DEBUG and u == 0:
                    nc.sync.dma_start(t["dbg_f"][:, sl], f_row)

                # Udiff = U1 - f*U2   (all [128, CH])
                u2f = work.tile([P, CH], F32, tag="u2f")
                nc.vector.tensor_tensor(u2f, U2, Frep[:, 0, :], Alu.mult)
                ud = udbuf[:, sl]
                nc.vector.tensor_tensor(ud, U1, u2f, Alu.subtract)

                # mean-square row for rmsinv, with the exact-eps fold:
                # ud is s1-scaled; mean(o^2)+eps = (ssq + 128*eps*s1^2)/128/s1^2
                # and the s1^2 cancels inside the normalize.
                sq = work.tile([P, CH], BF16, tag="sq")
                nc.vector.tensor_mul(sq, ud, ud)
                ssq = pb.tile([P, CH], F32, tag="pb", name=f"ssq_{u}_{nch}")
                nc.tensor.matmul(ssq[0:1, :], lhsT=ones_hot, rhs=sq)
                if not SKIP_ROWS:
                    sq1 = rows.tile([1, CH], F32, tag="row", name="sq1")
                    nc.vector.tensor_tensor(sq1, s1r, s1r, Alu.mult)
                    nc.vector.scalar_tensor_tensor(
                        out=adjb[0:1, sl],
                        in0=sq1,
                        scalar=float(P) * EPS,
                        in1=ssq[0:1, :],
                        op0=Alu.mult,
                        op1=Alu.add,
                    )
                else:
                    nc.vector.tensor_copy(adjb[0:1, sl], ssq[0:1, :])

            # per-unit batched rmsinv: one Ln + one Exp (same activation
            # table set stays loaded across the unit's 128 exp calls).
            lnr = rowsN.tile([1, N], F32, tag="rowN", name="lnr")
            nc.scalar.activation(lnr, adjb, Act.Ln, scale=1.0 / P)
            rir = rowsN.tile([1, N], F32, tag="rowN", name="rir")
            nc.scalar.activation(rir, lnr, Act.Exp, scale=-0.5)
            RrepN = repsN.tile([P, 1, N], F32, tag="repN", name="RrepN")
            nc.gpsimd.partition_broadcast(RrepN[:, 0, :], rir, channels=P)
            if DEBUG and u == 0:
                nc.sync.dma_start(t["dbg_ud"], udbuf)
                nc.sync.dma_start(t["dbg_rinv"], rir)
            # outn^T = (ud * geff) * rmsinv
            nc.vector.scalar_tensor_tensor(
                out=outnT[u],
                in0=udbuf,
                scalar=params_sb[:, 6:7],
                in1=RrepN[:, 0, :],
                op0=Alu.mult,
                op1=Alu.mult,
            )

        if DEBUG:
            nc.sync.dma_start(t["dbg_onT"], outnT[0])
            nc.sync.dma_start(t["dbg_onT1"], outnT[1])

        # ---- phase 3: output projection, y^T partial [DIM, N] ----
        for dt in ([] if ABL == "nop3" else range(DIM // P)):
            dsl = slice(dt * P, (dt + 1) * P)
            for nch in range(NCH):
                sl = slice(nch * CH, (nch + 1) * CH)
                yp = pb.tile([P, CH], F32, tag="pb", name=f"y_{dt}_{nch}")
                for u in range(PAIRS):
                    nc.tensor.matmul(
                        yp,
                        lhsT=_r(wo_sb[:, u, dsl]),
                        rhs=_r(outnT[u][:, sl]),
                        start=(u == 0),
                        stop=(u == PAIRS - 1),
                    )
                ysb = ypool.tile([P, CH], F32, tag="ysb")
                if (dt + nch) % 2 == 0:
                    nc.vector.tensor_copy(ysb, yp)
                else:
                    nc.scalar.copy(ysb, yp)
                nc.sync.dma_start(t["ypart"][dsl, sl], ysb)


def build_program(iters=1):
    nc = bacc.Bacc("TRN2", target_bir_lowering=False, debug=False)
    t = {
        "xt": nc.dram_tensor("xt", [DIM, N], BF16, kind="ExternalInput")[:],
        "wq": nc.dram_tensor("wq", [DIM, PAIRS * D2], BF16, kind="ExternalInput")[:],
        "wk": nc.dram_tensor("wk", [DIM, PAIRS * D2], BF16, kind="ExternalInput")[:],
        "wv": nc.dram_tensor("wv", [DIM, PAIRS * D2], BF16, kind="ExternalInput")[:],
        "wo": nc.dram_tensor("wo", [PAIRS * D2, DIM], BF16, kind="ExternalInput")[:],
        "params": nc.dram_tensor("params", [P, 8], F32, kind="ExternalInput")[:],
        "ypart": nc.dram_tensor("ypart", [DIM, N], F32, kind="ExternalOutput")[:],
    }
    if DEBUG:
        for nm, shp in [
            ("dbg_qt", [P, N]), ("dbg_kt", [P, N]), ("dbg_v", [P, MT * P]),
            ("dbg_f", [1, N]), ("dbg_rinv", [1, N]), ("dbg_ud", [P, N]),
            ("dbg_onT", [P, N]), ("dbg_onT1", [P, N]),
        ]:
            t[nm] = nc.dram_tensor(nm, shp, F32, kind="ExternalOutput")[:]
    with tile.TileContext(nc) as tc:
        _emit(tc, t, iters=iters)
    nc.compile()
    return nc


_NC_CACHE = {}


def _get_nc(iters=1):
    if iters not in _NC_CACHE:
        _NC_CACHE[iters] = build_program(iters)
    return _NC_CACHE[iters]


def make_core_inputs(x, Wq, bq, Wk, bk, Wv, bv, Wo, bo, g, lq1, lk1, lq2, lk2):
    """Host-side shard prep.  Returns (in_maps, lam) for the 8 cores."""
    x = np.asarray(x, np.float32)
    scaling = np.float32(HEAD_DIM**-0.5)
    lam1 = np.exp(np.sum(np.asarray(lq1, np.float32) * np.asarray(lk1, np.float32)))
    lam2 = np.exp(np.sum(np.asarray(lq2, np.float32) * np.asarray(lk2, np.float32)))
    lam = np.float32(lam1 - lam2 + LAMBDA_INIT)

    xt = np.ascontiguousarray(np.transpose(x, (0, 2, 1)))  # (B, DIM, N)
    Wq_s = np.asarray(Wq, np.float32) * scaling
    bq_s = np.asarray(bq, np.float32) * scaling
    geff = np.ascontiguousarray(
        (np.asarray(g, np.float32) * np.float32(1.0 - LAMBDA_INIT)).reshape(P, 1)
    )

    in_maps = []
    for c in range(CORES):
        b = c // 4
        grp = c % 4
        cols = slice(grp * PAIRS * D2, (grp + 1) * PAIRS * D2)
        params = np.zeros((P, 8), np.float32)
        params[:, 0:2] = bq_s[cols].reshape(PAIRS, P).T
        params[:, 2:4] = np.asarray(bk, np.float32)[cols].reshape(PAIRS, P).T
        params[:, 4:6] = np.asarray(bv, np.float32)[cols].reshape(PAIRS, P).T
        params[:, 6] = geff[:, 0]
        params[:, 7] = lam
        in_maps.append(
            {
                "xt": np.ascontiguousarray(xt[b]).astype(ml_dtypes.bfloat16),
                "wq": np.ascontiguousarray(Wq_s[:, cols]).astype(ml_dtypes.bfloat16),
                "wk": np.ascontiguousarray(np.asarray(Wk, np.float32)[:, cols]).astype(
                    ml_dtypes.bfloat16
                ),
                "wv": np.ascontiguousarray(np.asarray(Wv, np.float32)[:, cols]).astype(
                    ml_dtypes.bfloat16
                ),
                "wo": np.ascontiguousarray(np.asarray(Wo, np.float32)[cols, :]).astype(
                    ml_dtypes.bfloat16
                ),
                "params": params,
            }
        )
    return in_maps, lam


def gather_output(results, bo):
    """Sum per-core y^T partials per batch, transpose, add bo."""
    bo = np.asarray(bo, np.float32)
    out = np.empty((B, N, DIM), np.float32)
    for b in range(B):
        acc = np.zeros((DIM, N), np.float32)
        for c in range(b * 4, b * 4 + 4):
            acc += results[c]["ypart"]
        out[b] = acc.T + bo
    return out


_IN_CACHE = {}


def kernel(**inputs):
    from concourse.bass_utils import run_bass_kernel_spmd

    key = id(inputs.get("x"))
    if key in _IN_CACHE:
        in_maps = _IN_CACHE[key]
    else:
        in_maps, _ = make_core_inputs(**inputs)
        _IN_CACHE.clear()
        _IN_CACHE[key] = in_maps
    iters = int(os.environ.get("KERNEL_ITERS", "1"))
    nc = _get_nc(iters)
    trace = bool(int(os.environ.get("KERNEL_TRACE", "0")))
    res = run_bass_kernel_spmd(
        nc, in_maps, core_ids=list(range(CORES)), trace=trace
    )
    if trace and res.exec_time_ns is not None:
        print(f"HW exec time: {res.exec_time_ns} ns")
        kernel.last_exec_time_ns = res.exec_time_ns
        kernel.last_trace = res.instructions_and_trace
    return gather_output(res.results, inputs["bo"])


# ---------------- dev helpers (not used by the grading harness) ----------------


def _numpy_core_partial(im):
    """Reference computation of one core's ypart from its sharded inputs."""
    xt = im["xt"].astype(np.float64)  # [DIM, N]
    x = xt.T
    pr = im["params"]
    lam = float(pr[0, 7])
    ypart = np.zeros((DIM, N))
    for u in range(PAIRS):
        usl = slice(u * D2, (u + 1) * D2)
        q = x @ im["wq"][:, usl].astype(np.float64) + pr[:, u]  # [N, 128]
        k = x @ im["wk"][:, usl].astype(np.float64) + pr[:, 2 + u]
        v = x @ im["wv"][:, usl].astype(np.float64) + pr[:, 4 + u]
        s1 = q[:, :64] @ k[:, :64].T
        s2 = q[:, 64:] @ k[:, 64:].T
        p1 = np.exp(s1)
        p1 /= p1.sum(-1, keepdims=True)
        p2 = np.exp(s2)
        p2 /= p2.sum(-1, keepdims=True)
        diff = p1 - lam * p2
        o = diff @ v  # [N, 128]
        rms = 1.0 / np.sqrt((o * o).mean(-1, keepdims=True) + EPS)
        o = o * rms * pr[:, 6]
        ypart += im["wo"][usl, :].astype(np.float64).T @ o.T
    return ypart


if __name__ == "__main__":
    # CoreSim single-core numerical check:  python kernel.py sim
    mode = sys.argv[1] if len(sys.argv) > 1 else "sim"
    sys.path.insert(0, "/root/problem")
    import reference

    inputs = {k: np.asarray(v) for k, v in reference.setup_inputs().items()}
    in_maps, lam = make_core_inputs(**inputs)
    print("lam =", lam)
    nc = _get_nc()
    print("program built:", len(nc.inst_map) if hasattr(nc, "inst_map") else "?")
    if mode == "sim":
        from concourse.bass_interp import CoreSim

        sim = CoreSim(nc)
        for k, v in in_maps[0].items():
            sim.tensor(k)[:] = v
        sim.simulate()
        got = np.array(sim.tensor("ypart"))
        want = _numpy_core_partial(in_maps[0])
        err = np.abs(got - want)
        scale = np.abs(want).max()
        print("absmax err:", err.max(), "rel:", err.max() / scale, "scale:", scale)
        try:
            print("sim predicted time:", sim.time, "ns")
        except Exception as e:
            print("no sim time:", e)
